# revision 32
# baseline (speedup 1.0000x reference)
"""FermiNet forward + slogdet kernel for 8 Trainium2 NeuronCores.

Data-parallel: batch (walker) dim sharded 8 ways, weights replicated.
Per core: 256 walkers -> 8192 electron rows.

Pipeline per core (fp32 data, fp32r matmuls):
  F1T[h0, r]  = tanh(We1^T X^T + be1)          PE (K=3) + ACT
  F2T[h1, r]  = tanh(We2^T F1T + be2)          PE (K=256) + ACT
  hT_p[(olo,k), r] = tanh(Wo1pair^T F2T + bo1) PE (K=256, 16 orbital pairs) + ACT
  M[o, r]     = sum_p Wo2bd_p^T hT_p + bo2     PE (zero-padded block-diag columns,
                                               all 16 pairs accumulate into one
                                               [32, 512] PSUM tile) + DVE bias add
  transpose   Msb [32,512] -> [128,32] x4 via PE transpose, DVE copy, then one
              DMA per 4 walkers into Mge[w, n*32+o] (n-major Slater matrices,
              walkers on partitions)
  GE (no pivoting) on Mge batches of 128 walkers: logabsdet = sum ln |diag|.

Scheduling (v3):
  - Head: xt chunk0 + we1 are DMA'd first; all small per-partition
    constants are packed into two tensors (smallc [128,20], eyebo [32,33])
    to avoid 256x8B descriptor storms ahead of the first matmul; wo1 is
    split in 4 and issued on the ACT DMA queue in parallel with the SP
    queue.
  - GE batch 0 (walkers 0-127) is interleaved between chunks 8..15, with
    recip+fac on DVE and the full trailing update on GPSIMD (idle
    mid-phase), so the DVE never delays the per-chunk msb/transpose work
    that recycles PSUM for the PE.
  - GE batch 1 runs after the last chunk (exposed tail): DVE takes
    recip, fac and the leading columns of each update, GPSIMD a fixed
    trailing column block [GP_S..31]; an 8-deep fac pool lets DVE run
    ahead of GPSIMD.
"""

import numpy as np

B = 2048
NE = 32
NO = 32
H0 = 256
H1 = 256
H2 = 64
NCORES = 8
BC = B // NCORES          # walkers per core
R = BC * NE               # electron rows per core
RC = 512                  # rows per chunk
NCHUNK = R // RC          # 16
WPC = RC // NE            # walkers per chunk = 16
NPAIR = NO // 2           # 16
GE_BATCH = 128            # walkers per GE batch
NBATCH = BC // GE_BATCH   # 2
CHUNKS_PER_BATCH = NCHUNK // NBATCH  # 8

GP_S1 = 25                # tail GE batch 1: GPSIMD owns columns [GP_S1..31]
USE_DIVIDE = False        # divide ALU op fails NEFF lowering; use reciprocal+mul

_cache = {}


def _pack_weights(We1, be1, We2, be2, Wo1, bo1, Wo2, bo2):
    """Host-side weight packing into kernel layouts (all fp32)."""
    f32 = np.float32
    we1 = np.ascontiguousarray(We1, dtype=f32)
    we2 = np.zeros((128, 4 * 128), dtype=f32)
    for q in range(2):
        for h in range(2):
            we2[:, (q * 2 + h) * 128 : (q * 2 + h + 1) * 128] = We2[
                q * 128 : (q + 1) * 128, h * 128 : (h + 1) * 128
            ]
    wo1 = np.zeros((128, NPAIR * 2 * 128), dtype=f32)
    for pr in range(NPAIR):
        for q in range(2):
            col = np.concatenate(
                [
                    Wo1[2 * pr, q * 128 : (q + 1) * 128, :],
                    Wo1[2 * pr + 1, q * 128 : (q + 1) * 128, :],
                ],
                axis=1,
            )
            wo1[:, (pr * 2 + q) * 128 : (pr * 2 + q + 1) * 128] = col
    wo2bd = np.zeros((128, NPAIR * 32), dtype=f32)
    for pr in range(NPAIR):
        for olo in range(2):
            o = 2 * pr + olo
            wo2bd[olo * 64 : olo * 64 + 64, pr * 32 + o] = Wo2[o]
    # small per-partition constants packed into one [128, 20] tensor:
    # cols 0:2 = be1p, 2:4 = be2p, 4:20 = bo1p
    smallc = np.zeros((128, 20), dtype=f32)
    smallc[:, 0:2] = be1.reshape(2, 128).T
    smallc[:, 2:4] = be2.reshape(2, 128).T
    for pr in range(NPAIR):
        smallc[:64, 4 + pr] = bo1[2 * pr]
        smallc[64:, 4 + pr] = bo1[2 * pr + 1]
    # eye32 + bo2 packed into one [32, 33] tensor
    eyebo = np.zeros((32, 33), dtype=f32)
    eyebo[:, :32] = np.eye(32, dtype=f32)
    eyebo[:, 32] = bo2
    return {
        "we1": we1,
        "we2": we2,
        "wo1": wo1,
        "wo2bd": wo2bd,
        "smallc": smallc,
        "eyebo": eyebo,
    }


def build_kernel(enable_asserts=False):
    """Build and compile the Bass module (single NEFF, run SPMD on 8 cores)."""
    import concourse.bacc as bacc
    import concourse.mybir as mybir
    import concourse.tile as tile

    f32 = mybir.dt.float32
    f32r = mybir.dt.float32r
    Tanh = mybir.ActivationFunctionType.Tanh
    Abs = mybir.ActivationFunctionType.Abs
    Ln = mybir.ActivationFunctionType.Ln
    mult = mybir.AluOpType.mult
    sub = mybir.AluOpType.subtract
    div = mybir.AluOpType.divide

    nc = bacc.Bacc(
        "TRN2",
        target_bir_lowering=False,
        debug=False,
        enable_asserts=enable_asserts,
    )

    xt_d = nc.dram_tensor("xt", [3, R], f32r, kind="ExternalInput").ap()
    we1_d = nc.dram_tensor("we1", [3, 256], f32r, kind="ExternalInput").ap()
    we2_d = nc.dram_tensor("we2", [128, 512], f32r, kind="ExternalInput").ap()
    wo1_d = nc.dram_tensor("wo1", [128, NPAIR * 256], f32r, kind="ExternalInput").ap()
    wo2bd_d = nc.dram_tensor("wo2bd", [128, NPAIR * 32], f32r, kind="ExternalInput").ap()
    smallc_d = nc.dram_tensor("smallc", [128, 20], f32, kind="ExternalInput").ap()
    eyebo_d = nc.dram_tensor("eyebo", [32, 33], f32, kind="ExternalInput").ap()
    out_d = nc.dram_tensor("out", [BC], f32, kind="ExternalOutput").ap()

    with tile.TileContext(nc) as tc:
        with (
            tc.tile_pool(name="const", bufs=1) as cpool,
            tc.tile_pool(name="act", bufs=6) as apool,
            tc.tile_pool(name="hbuf", bufs=8) as hpool,
            tc.tile_pool(name="msb", bufs=3) as mpool,
            tc.tile_pool(name="sb4p", bufs=3) as s4pool,
            tc.tile_pool(name="mge", bufs=2) as gepool,
            tc.tile_pool(name="ges", bufs=3) as spool,
            tc.tile_pool(name="mm", bufs=5, space="PSUM") as pe_psum,
            tc.tile_pool(name="acc", bufs=2, space="PSUM") as m_psum,
            tc.tile_pool(name="ptp", bufs=1, space="PSUM") as t_psum,
        ):
            # ---- resident constants / inputs, ordered by first use.
            # SP queue: chunk-0 x first, then small constants; ACT queue
            # (fans out across DMA engines): we2, wo2bd, then wo1 in 4
            # parts.
            xt = cpool.tile([3, R], f32r)
            nc.sync.dma_start(xt[:, :RC], xt_d[:, :RC])
            we1 = cpool.tile([3, 256], f32r)
            nc.sync.dma_start(we1[:], we1_d[:])
            we2 = cpool.tile([128, 512], f32r)
            nc.sync.dma_start(we2[:, :128], we2_d[:, :128])
            nc.gpsimd.dma_start(we2[:, 128:256], we2_d[:, 128:256])

            # small per-partition constants: many tiny descriptors, so keep
            # them off the queues that feed the first matmuls
            smallc = cpool.tile([128, 20], f32)
            nc.gpsimd.dma_start(smallc[:], smallc_d[:])
            eyebo = cpool.tile([32, 33], f32)
            nc.gpsimd.dma_start(eyebo[:], eyebo_d[:])

            nc.scalar.dma_start(we2[:, 256:384], we2_d[:, 256:384])
            nc.scalar.dma_start(we2[:, 384:], we2_d[:, 384:])
            wo2bd = cpool.tile([128, NPAIR * 32], f32r)
            nc.scalar.dma_start(wo2bd[:], wo2bd_d[:])
            wo1 = cpool.tile([128, NPAIR * 256], f32r)
            for part in range(4):
                sl = slice(part * 1024, (part + 1) * 1024)
                nc.scalar.dma_start(wo1[:, sl], wo1_d[:, sl])
            # bulk x rows: not needed until chunk 1 (~29us) — keep them off
            # the head critical path (chunk-0 F2 stalls on DMA contention)
            nc.sync.dma_start(xt[:, RC : 8 * RC], xt_d[:, RC : 8 * RC])
            nc.sync.dma_start(xt[:, 8 * RC :], xt_d[:, 8 * RC :])

            be1p = smallc[:, 0:2]
            be2p = smallc[:, 2:4]
            bo1p = smallc[:, 4:20]
            eye32 = eyebo[:, 0:32]
            bo2t = eyebo[:, 32:33]

            mge = [
                gepool.tile([GE_BATCH, NE * NO], f32, tag="mge", name=f"mge{b}")
                for b in range(NBATCH)
            ]
            # private GPSIMD copy of columns [GP_S1..31] for the tail batch:
            # avoids tile-granular false dependencies between DVE and GPSIMD
            WG1 = NE - GP_S1
            mgeg1 = gepool.tile([GE_BATCH, NE * WG1], f32, tag="mgeg", name="mgeg1")

            # private GPSIMD copy for batch 0's block [GP_S0..31]
            GP_S0 = 21
            WG0 = NE - GP_S0
            mgeg0 = gepool.tile([GE_BATCH, NE * WG0], f32, tag="mgeg0", name="mgeg0")

            def ge_prep0():
                mv = mge[0][:].rearrange("p (n o) -> p n o", o=NO)
                mvg = mgeg0[:].rearrange("p (n o) -> p n o", o=WG0)
                nc.vector.tensor_copy(mvg[:, :, :], mv[:, :, GP_S0:])

            def ge_step0(k):
                """GE step for the hidden batch 0 (interleaved in chunks
                8..15): baseline-proven split — DVE cols k+1..GP_S0-1,
                GPSIMD cols [GP_S0..31] in its private mgeg0 copy."""
                mv = mge[0][:].rearrange("p (n o) -> p n o", o=NO)
                mvg = mgeg0[:].rearrange("p (n o) -> p n o", o=WG0)
                m = NE - 1 - k
                recip = spool.tile([GE_BATCH, 1], f32, tag="recip")
                nc.vector.reciprocal(recip[:], mv[:, k, k : k + 1])
                use_gp = k <= GP_S0 - 3
                fac = spool.tile([GE_BATCH, NE], f32, tag="fac")
                nc.vector.tensor_scalar_mul(fac[:, :m], mv[:, k + 1 :, k], recip[:])
                if k == GP_S0 - 2:
                    nc.vector.tensor_copy(mv[:, k:, GP_S0:], mvg[:, k:, :])
                jD1 = GP_S0 if use_gp else NE
                wD = jD1 - (k + 1)
                if wD > 0:
                    tmp = spool.tile([GE_BATCH, (NE - 1) * (NE - 1)], f32, tag="tmp0")
                    tview = tmp[:, : m * wD].rearrange("p (i j) -> p i j", i=m)
                    nc.vector.tensor_tensor(
                        tview,
                        fac[:, :m].unsqueeze(2).broadcast_to([GE_BATCH, m, wD]),
                        mv[:, k : k + 1, k + 1 : jD1].broadcast_to([GE_BATCH, m, wD]),
                        mult,
                    )
                    nc.vector.tensor_tensor(
                        mv[:, k + 1 :, k + 1 : jD1],
                        mv[:, k + 1 :, k + 1 : jD1],
                        tview,
                        sub,
                    )
                if use_gp:
                    tmp2 = spool.tile([GE_BATCH, (NE - 1) * WG0], f32, tag="tmp02")
                    t2 = tmp2[:, : m * WG0].rearrange("p (i j) -> p i j", i=m)
                    nc.gpsimd.tensor_tensor(
                        t2,
                        fac[:, :m].unsqueeze(2).broadcast_to([GE_BATCH, m, WG0]),
                        mvg[:, k : k + 1, :].broadcast_to([GE_BATCH, m, WG0]),
                        mult,
                    )
                    nc.gpsimd.tensor_tensor(
                        mvg[:, k + 1 :, :],
                        mvg[:, k + 1 :, :],
                        t2,
                        sub,
                    )

            def ge_prep1():
                """Copy columns [GP_S1..31] of batch 1 into GPSIMD's private
                tile (values identical; breaks DVE<->GPSIMD tile coupling)."""
                mv = mge[1][:].rearrange("p (n o) -> p n o", o=NO)
                mvg = mgeg1[:].rearrange("p (n o) -> p n o", o=WG1)
                nc.vector.tensor_copy(mvg[:, :, :], mv[:, :, GP_S1:])

            def ge_step1(k):
                """GE step for the exposed tail batch 1 (latency-critical).

                k <= GP_S1-3: DVE handles columns k+1..GP_S1-1 in mge,
                GPSIMD columns [GP_S1..31] in its private mgeg1 copy.
                k == GP_S1-2: DVE copies mgeg1 back into mge (single
                cross-engine handoff) and does the full-width update.
                Fused product: tmp = (col * recip) * row in one STT op."""
                mv = mge[1][:].rearrange("p (n o) -> p n o", o=NO)
                mvg = mgeg1[:].rearrange("p (n o) -> p n o", o=WG1)
                m = NE - 1 - k
                recip = spool.tile([GE_BATCH, 1], f32, tag="recip")
                nc.vector.reciprocal(recip[:], mv[:, k, k : k + 1])
                use_gp = k <= GP_S1 - 3
                if use_gp:
                    # GPSIMD can't use an AP scalar (Pool STT unsupported);
                    # give it grr = recip * (its private pivot row): only
                    # WG1 elems on the DVE chain instead of an m-wide fac
                    grr = spool.tile([GE_BATCH, WG1], f32, tag="grr")
                    nc.vector.tensor_scalar_mul(grr[:], mvg[:, k, :], recip[:])
                if k == GP_S1 - 2:
                    # hand back GPSIMD's block: rows k.. cols GP_S1..31
                    nc.vector.tensor_copy(mv[:, k:, GP_S1:], mvg[:, k:, :])
                jD1 = GP_S1 if use_gp else NE
                wD = jD1 - (k + 1)
                if wD > 0:
                    tmp = spool.tile([GE_BATCH, (NE - 1) * (NE - 1)], f32, tag="tmp")
                    tview = tmp[:, : m * wD].rearrange("p (i j) -> p i j", i=m)
                    # fused (col * recip) * row: one STT op, no fac tensor
                    nc.vector.scalar_tensor_tensor(
                        tview,
                        mv[:, k + 1 :, k]
                        .unsqueeze(2)
                        .broadcast_to([GE_BATCH, m, wD]),
                        recip[:],
                        mv[:, k : k + 1, k + 1 : jD1].broadcast_to(
                            [GE_BATCH, m, wD]
                        ),
                        mult,
                        mult,
                    )
                    nc.vector.tensor_tensor(
                        mv[:, k + 1 :, k + 1 : jD1],
                        mv[:, k + 1 :, k + 1 : jD1],
                        tview,
                        sub,
                    )
                if use_gp:
                    tmp2 = spool.tile([GE_BATCH, (NE - 1) * WG1], f32, tag="tmp2")
                    t2 = tmp2[:, : m * WG1].rearrange("p (i j) -> p i j", i=m)
                    nc.gpsimd.tensor_tensor(
                        t2,
                        mv[:, k + 1 :, k].unsqueeze(2).broadcast_to(
                            [GE_BATCH, m, WG1]
                        ),
                        grr[:].unsqueeze(1).broadcast_to([GE_BATCH, m, WG1]),
                        mult,
                    )
                    nc.gpsimd.tensor_tensor(
                        mvg[:, k + 1 :, :],
                        mvg[:, k + 1 :, :],
                        t2,
                        sub,
                    )

            def ge_finish(b):
                """logabsdet from the GE diagonal of batch b."""
                diag = mge[b][:, 0 : NE * NO : NO + 1]  # [128, 32] stride 33
                absd = spool.tile([GE_BATCH, NE], f32, tag="absd")
                nc.scalar.activation(absd[:], diag, Abs)
                lnd = spool.tile([GE_BATCH, NE], f32, tag="lnd")
                ld = spool.tile([GE_BATCH, 1], f32, tag="ld")
                nc.scalar.activation(lnd[:], absd[:], Ln, accum_out=ld[:])
                nc.sync.dma_start(out_d[b * GE_BATCH : (b + 1) * GE_BATCH], ld[:])

            # GE batch 0 steps interleaved into chunks 8..15: 31 steps,
            # front-loaded (the late steps are cheap, and the k=GP_S-2
            # handoff should find GPSIMD already done).
            _bnd = [0, 5, 10, 14, 18, 22, 25, 28, 31]
            ge0_sched = [range(_bnd[i], _bnd[i + 1]) for i in range(8)]

            # ---- M-stage (msb add, PE transposes, sb4 copy, mge DMAs) is
            # software-pipelined one chunk behind the matmul stream: chunk
            # c-1's M-stage is emitted in the middle of chunk c's pair loop,
            # where msb has ~4us of slack instead of sitting on the chunk
            # turn's critical path (PE stalls there also drop the PE clock
            # out of its max p-state).
            def emit_mstage(st):
                mps, b, w0, last = st
                msb = mpool.tile([NO, RC], f32, tag="msb")
                # always ACT: chunks whose M-stage ran on ACT measured 1.1us
                # faster than the DVE variant, even with an idle DVE
                nc.scalar.add(msb[:], mps[:], bo2t)
                ptr = t_psum.tile([128, 128], f32, tag="ptp")
                for t4 in range(4):
                    nc.tensor.transpose(
                        ptr[:, t4 * 32 : (t4 + 1) * 32],
                        msb[:, t4 * 128 : (t4 + 1) * 128],
                        eye32,
                    )
                sb4 = s4pool.tile([128, 128], f32, tag="sb4")
                nc.vector.tensor_copy(sb4[:], ptr[:])
                last_engs = [nc.sync, nc.scalar, nc.gpsimd, nc.sync]
                for t4 in range(4):
                    # last chunk: spread across queues to shorten the
                    # handoff into the exposed batch-1 GE
                    eng = last_engs[t4] if last else nc.sync
                    eng.dma_start(
                        mge[b][w0 + t4 * 4 : w0 + t4 * 4 + 4, :],
                        sb4[:, t4 * 32 : (t4 + 1) * 32],
                    )

            mstage_prev = None

            for c in range(NCHUNK):
                rs = c * RC
                xs = xt[:, rs : rs + RC]
                # ---- F1: [3,128] x [3,512] -> psum [128,512] x2 halves
                f1 = []
                for q in range(2):
                    pf = pe_psum.tile([128, RC], f32, tag="mm")
                    nc.tensor.matmul(
                        pf[:],
                        we1[:, q * 128 : (q + 1) * 128],
                        xs,
                        start=True,
                        stop=True,
                    )
                    t = apool.tile([128, RC], f32r, tag="f1")
                    nc.scalar.activation(t[:], pf[:], Tanh, bias=be1p[:, q : q + 1])
                    f1.append(t)
                # ---- F2: accumulate over q
                f2 = []
                for h in range(2):
                    pf = pe_psum.tile([128, RC], f32, tag="mm")
                    for q in range(2):
                        nc.tensor.matmul(
                            pf[:],
                            we2[:, (q * 2 + h) * 128 : (q * 2 + h + 1) * 128],
                            f1[q][:],
                            start=(q == 0),
                            stop=(q == 1),
                        )
                    t = apool.tile([128, RC], f32r, tag="f2")
                    nc.scalar.activation(t[:], pf[:], Tanh, bias=be2p[:, h : h + 1])
                    f2.append(t)
                # ---- orbital pairs + Wo2 accumulation
                mps = m_psum.tile([NO, RC], f32, tag="acc")
                for pr in range(NPAIR):
                    if pr == 2 and mstage_prev is not None:
                        emit_mstage(mstage_prev)
                        mstage_prev = None
                    po = pe_psum.tile([128, RC], f32, tag="mm")
                    for q in range(2):
                        nc.tensor.matmul(
                            po[:],
                            wo1[:, (pr * 2 + q) * 128 : (pr * 2 + q + 1) * 128],
                            f2[q][:],
                            start=(q == 0),
                            stop=(q == 1),
                        )
                    ht = hpool.tile([128, RC], f32r, tag="h")
                    nc.scalar.activation(ht[:], po[:], Tanh, bias=bo1p[:, pr : pr + 1])
                    nc.tensor.matmul(
                        mps[:],
                        wo2bd[:, pr * 32 : (pr + 1) * 32],
                        ht[:],
                        start=(pr == 0),
                        stop=(pr == NPAIR - 1),
                    )
                # ---- defer this chunk's M-stage into the next chunk's
                # pair loop (emitted after the loop for the last chunk)
                b = c // CHUNKS_PER_BATCH
                w0 = (c % CHUNKS_PER_BATCH) * WPC
                mstage_prev = (mps, b, w0, c == NCHUNK - 1)
                # ---- interleaved GE batch 0 steps during chunks 8..15
                if c >= CHUNKS_PER_BATCH:
                    if c == CHUNKS_PER_BATCH:
                        ge_prep0()
                    for k in ge0_sched[c - CHUNKS_PER_BATCH]:
                        ge_step0(k)

            # ---- last chunk's M-stage, then the tail
            emit_mstage(mstage_prev)
            mstage_prev = None

            # ---- tail: finish(0) first so its Ln ACT-table load hides
            # under GE batch 1's DVE/GPSIMD steps; then batch 1 GE.
            # tile_wait_until pins finish(0) late in the STATIC schedule —
            # without it the greedy scheduler hoists the ABS/LN (plus two
            # 1.28us ACT-table loads) into the mid-kernel tanh stream as
            # soon as GE batch 0 completes, stalling ACT and the PE.
            ge_prep1()
            with tc.tile_wait_until(1.0):
                ge_finish(0)
            for k in range(NE - 1):
                ge_step1(k)
            with tc.tile_wait_until(2.0):
                ge_finish(1)

    nc.compile()
    return nc


def _get_nc():
    if "nc" not in _cache:
        _cache["nc"] = build_kernel()
    return _cache["nc"]


def _run(inputs, trace=False, trace_kwargs=None):
    from concourse.bass_utils import run_bass_kernel_spmd

    nc = _get_nc()
    packed = _pack_weights(
        inputs["We1"], inputs["be1"], inputs["We2"], inputs["be2"],
        inputs["Wo1"], inputs["bo1"], inputs["Wo2"], inputs["bo2"],
    )
    x = np.asarray(inputs["x"], dtype=np.float32)
    in_maps = []
    for c in range(NCORES):
        xs = x[c * BC : (c + 1) * BC].reshape(R, 3)
        in_map = dict(packed)
        in_map["xt"] = np.ascontiguousarray(xs.T)
        in_maps.append(in_map)
    kwargs = {}
    if trace:
        kwargs["trace"] = True
        if trace_kwargs:
            kwargs["trace_kwargs"] = trace_kwargs
    res = run_bass_kernel_spmd(nc, in_maps, core_ids=list(range(NCORES)), **kwargs)
    out = np.concatenate([res.results[c]["out"] for c in range(NCORES)])
    return out, res


def kernel(**inputs) -> np.ndarray:
    out, _ = _run(inputs)
    return out.astype(np.float32)

